# revision 1
# baseline (speedup 1.0000x reference)
"""Trainium2 Bass kernel for margin-ranking + weighted-BCE loss pair.

Math
----
reference margin part (binary labels l in {0,1}):
  S_full := sum_{i,j in [B]^2} relu(m - (p_i-p_j)(l_i-l_j))
          = (n0^2 + n1^2) * relu(m) + 2*S,
  S := sum_{i in P1, j in P0} relu(m - p_i + p_j),   P1={l=1}, P0={l=0}
  margin_loss = S_full/(2B) - relu(m)/2.

S is evaluated via a 32-knot piecewise-linear quadrature instead of the
full 16.7M-pair grid: with f(a) = sum_j v_j relu(p_j + m - a) convex PWL,
S = sum_i w_i f(a_i) ~= sum_k F_k * J_k, where F_k = f(g_k) on a uniform
grid g_k = (k-16)*5/16 and J_k is the linear-binning (hat)
histogram of the a_i = second difference of H(g) = sum_i w_i relu(a_i-g):
J_k = (H_{k-1} - 2 H_k + H_{k+1})/h. Interp error ~3e4 on S ~2e7 vs a
1.5e6 tolerance budget. sum_k J_k = n1 falls out for free.

BCE uses the same quadrature: the reference's log(e^-mv + e^-z-mv)+mv
is softplus(-z), so sum_i u_i*sp(-z_i) (u = 1 and u = t) =
dot(phi, hat-histogram of z) with phi_k = log(1+e^-g_k); no Exp/Ln
activations or table loads are needed; only sum z*(1-t) stays
elementwise. Host folds pos_weight into the partial combine.

Label/target masks fold into the relu arguments (u*relu(x) =
relu(x - C*(1-u)) for u in {0,1}, C=32). A device AllReduce measures
~60us here for 512 B, so H is REPLICATED (every core sums all B
columns; F/z keep per-core B/8 shards). With 32 knots, each 2-bank
PSUM tile holds THREE 32-partition column-sets (PE dst base partitions
are limited to {0,32,64}), so H is 3 consume chunks (padded to 9216
cols with mask-killed pads) and F/ZT half-chunks, split over DVE and
ScalarE. A single f32 stencil matmul with T' = tile(T32,(3,3)) folds
the partition groups and applies [1,-2,1] to every accumulator at
once; host combine rescales for the 3-way vs 2-way fold multiplicity.
"""

import numpy as np
import ml_dtypes

import concourse.bacc as bacc
import concourse.bass as bass
import concourse.bass_isa as bass_isa
import concourse.mybir as mybir
import concourse.tile as tile
from concourse.bass_utils import run_bass_kernel_spmd

B = 8192
NCORES = 8
SH = B // NCORES           # 1024 F/Z/ZT columns per core
G = 32                     # grid knots
HSTEP = 5.0 / 16.0         # grid spacing, bf16-exact
CMASK = 32.0               # mask offset, bf16-exact
P = 128
NG = 3                     # packed column-sets (PE dst base in {0,32,64})
HB = 512                   # one PSUM bank of f32
TCOLS = 2 * NG * HB        # columns covered by one [96, 2-bank] tile
HPAD = 9216                # H columns padded to 3 tiles (pads are masked)
NCH_H = HPAD // TCOLS      # 3 H chunks
BCE_N = B // NCORES        # 1024 -> [128, 8]
BCE_F = BCE_N // P         # 8

f32 = mybir.dt.float32
bf16 = mybir.dt.bfloat16
fp8 = mybir.dt.float8e4


def _grid():
    return (np.arange(G, dtype=np.float64) - G // 2) * HSTEP


def _build_program(margin: float):
    from contextlib import ExitStack

    m = float(margin)
    nc = bacc.Bacc("TRN2", target_bir_lowering=False, debug=False,
                   num_devices=NCORES)
    Relu = mybir.ActivationFunctionType.Relu
    add = mybir.AluOpType.add
    mult = mybir.AluOpType.mult
    amax = mybir.AluOpType.max

    # rhs columns: [0:P) lhsT coeffs | H | F | Z | ZT  (coeffs ride in the
    # same tensor so one DMA covers them: every extra dma_start costs ~16
    # descriptors serially on its ring). Only TWO data rows: the grid and
    # all constant terms live in the per-partition BIAS of the consume op,
    # so the input DMA is 45 KB instead of 112 KB.
    NC_ALL = P + HPAD + 2 * SH
    AUXW = 2 * BCE_F + 2 + 6
    rhs_d = nc.dram_tensor("rhs", [2, NC_ALL], bf16, kind="ExternalInput")
    aux_d = nc.dram_tensor("aux", [P, AUXW], f32, kind="ExternalInput")
    tmat_d = nc.dram_tensor("tmat", [P, P], f32, kind="ExternalInput")
    out_d = nc.dram_tensor("out", [1, 5], f32, kind="ExternalOutput")

    with tile.TileContext(nc) as tc, ExitStack() as ctx:
        small = ctx.enter_context(tc.tile_pool(name="small", bufs=1))
        scr = ctx.enter_context(tc.tile_pool(name="scr", bufs=2))
        psum = ctx.enter_context(
            tc.tile_pool(name="psum", bufs=4, space=bass.MemorySpace.PSUM))

        # ---- input loads: one DMA per ring (tmat trails on the scalar
        # ring; it is not needed until the stencil matmul) -----------------
        rhs_t = small.tile([2, NC_ALL], bf16, tag="rhs")
        aux_t = small.tile([P, AUXW], f32, tag="aux")
        tmat_t = small.tile([P, P], f32, tag="tmat")
        zt_ = aux_t[:, 0:BCE_F]
        tt = aux_t[:, BCE_F: 2 * BCE_F]
        phi_t = aux_t[:, 2 * BCE_F: 2 * BCE_F + 2]
        bias_h = aux_t[:, 2 * BCE_F + 2: 2 * BCE_F + 3]
        bias_f = aux_t[:, 2 * BCE_F + 3: 2 * BCE_F + 4]
        bias_z = aux_t[:, 2 * BCE_F + 4: 2 * BCE_F + 5]
        nbias_h = aux_t[:, 2 * BCE_F + 5: 2 * BCE_F + 6]
        nbias_z = aux_t[:, 2 * BCE_F + 7: 2 * BCE_F + 8]
        HALF = P + 2 * TCOLS        # coeffs + chunks 0-1x on the sync ring
        nc.sync.dma_start(out=rhs_t[:, 0:HALF], in_=rhs_d[:, 0:HALF])
        nc.scalar.dma_start(out=rhs_t[:, HALF:], in_=rhs_d[:, HALF:])
        nc.gpsimd.dma_start(out=aux_t[:, :], in_=aux_d[:, :])
        nc.gpsimd.dma_start(out=tmat_t[:, :], in_=tmat_d[:, :])

        ones1 = small.tile([P, 1], f32, tag="ones1")
        nc.gpsimd.memset(ones1[:, :], 1.0)

        # ---- BCE linear term sum z*(1-t) on GpSimd -----------------------
        tz = small.tile([P, BCE_F], f32, tag="tz")
        r2 = small.tile([P, BCE_F], f32, tag="r2")
        nc.gpsimd.tensor_mul(tz[:, :], tt[:, :], zt_[:, :])
        nc.gpsimd.tensor_sub(r2[:, :], zt_[:, :], tz[:, :])

        # ---- matmul/consume pipeline -------------------------------------
        # lhsT columns 0:64 carry the H/ZT coefficients, 64:128 the F/Z
        # ones; each [128, 2-bank] PSUM tile holds two 64-partition
        # column-sets. acc6 column map: 0,2,3 = DVE H chunks, 1 = ScalarE
        # H chunk, 4 = Z, 5 = ZT.
        acc6 = small.tile([P, 3], f32, tag="acc6")
        accs = small.tile([P, 3], f32, tag="accs")
        facc = small.tile([P, 1], f32, tag="facc")

        hcoef = rhs_t[:, 0:G]
        fcoef = rhs_t[:, G: 2 * G]
        G3 = NG * G
        nc.gpsimd.memset(acc6[:, :], 0.0)
        nc.gpsimd.memset(accs[:, :], 0.0)
        nc.gpsimd.memset(facc[:, :], 0.0)
        for c in range(NCH_H):
            pb = psum.tile([P, 2, HB], f32, tag="blk")
            for q in range(2 * NG):
                grp, bk = divmod(q, 2)
                col = P + TCOLS * c + HB * q
                nc.tensor.matmul(pb[G * grp: G * (grp + 1), bk, :], hcoef,
                                 rhs_t[:, col: col + HB],
                                 start=True, stop=True)
            if c == 1:
                sa = scr.tile([P, 2, HB], f32, tag="scr_a")
                nc.scalar.activation(sa[0:G3, :, :], pb[0:G3, :, :], Relu,
                                     bias=bias_h[0:G3, :],
                                     accum_out=acc6[0:G3, 1:2])
            else:
                sd = scr.tile([P, 2, HB], f32, tag="scr_d")
                nc.vector.tensor_scalar(sd[0:G3, :, :], pb[0:G3, :, :],
                                        nbias_h[0:G3, :], bias_h[0:G3, :],
                                        amax, add,
                                        accum_out=acc6[0:G3, 2 * (c // 2):
                                                       2 * (c // 2) + 1])

        # F + ZT share a tile: bank 0 = F cols, bank 1 = ZT cols. The ZT
        # bank (z + C*t) is consumed TWICE: bias -C-g keeps the t=1 kinks
        # (t=0 pushed below every knot), bias -g keeps the t=0 kinks (t=1
        # sits at z+C-g > 0 for every knot, i.e. exactly linear, and the
        # [1,-2,1] stencil annihilates it). No separate Z columns needed.
        pfz = psum.tile([P, 2, HB], f32, tag="blk")
        for hi in range(2):
            col = P + HPAD + HB * hi
            nc.tensor.matmul(pfz[G * hi: G * (hi + 1), 0, :], fcoef,
                             rhs_t[:, col: col + HB],
                             start=True, stop=True)
        for hi in range(2):
            col = P + HPAD + SH + HB * hi
            nc.tensor.matmul(pfz[G * hi: G * (hi + 1), 1, :], hcoef,
                             rhs_t[:, col: col + HB],
                             start=True, stop=True)

        G2 = 2 * G
        sf = scr.tile([P, 1, HB], f32, tag="scr_a")
        nc.scalar.activation(sf[0:G2, :, :], pfz[0:G2, 0:1, :], Relu,
                             bias=bias_f[0:G2, :],
                             accum_out=facc[0:G2, 0:1])
        sz = scr.tile([P, 1, HB], f32, tag="scr_z")
        nc.scalar.activation(sz[0:G2, :, :], pfz[0:G2, 1:2, :], Relu,
                             bias=bias_z[0:G2, :],
                             accum_out=accs[0:G2, 1:2])
        # pre-sum the H accumulators while F/ZT still stream
        nc.vector.tensor_reduce(accs[:, 0:1], acc6[:, :],
                                axis=mybir.AxisListType.X, op=add)
        szt = scr.tile([P, 1, HB], f32, tag="scr_d")
        nc.vector.tensor_scalar(szt[0:G2, :, :], pfz[0:G2, 1:2, :],
                                nbias_h[0:G2, :], bias_h[0:G2, :],
                                amax, add, accum_out=accs[0:G2, 2:3])

        # ---- fold + stencil on all six accumulators at once --------------
        # T' = tile(T64, (2, 2)): pd2[m, n] = D2(acc_n)[m mod 64], summing
        # both partition halves of each accumulator in the same matmul.
        pd2 = psum.tile([P, 3], f32, tag="blk")
        nc.tensor.matmul(pd2[:, :], tmat_t[:, :], accs[:, :],
                         start=True, stop=True)

        # stacked: [dot-partial, h*n1-ish, sp1, spt, zlin]
        stacked = small.tile([P, 5], f32, tag="stacked")
        nc.vector.tensor_reduce(stacked[:, 4:5], r2[:, :],
                                axis=mybir.AxisListType.X, op=add)
        nc.vector.tensor_copy(stacked[:, 1:2], pd2[:, 0:1])
        nc.vector.tensor_tensor(stacked[:, 0:1], facc[:, :], pd2[:, 0:1],
                                op=mult)
        nc.vector.tensor_tensor(stacked[:, 2:4], phi_t[:, 0:2], pd2[:, 1:3],
                                op=mult)

        pfin = psum.tile([1, 5], f32, tag="blk")
        nc.tensor.matmul(pfin[:, :], ones1[:, :], stacked[:, :],
                         start=True, stop=True)
        outt = small.tile([1, 5], f32, tag="outt")
        nc.vector.tensor_copy(outt[:, :], pfin[:, :])
        nc.sync.dma_start(out=out_d[:, :], in_=outt[:, :])

    nc.compile()
    return nc


_programs: dict = {}


def _get_program(margin: float):
    key = margin
    if key not in _programs:
        _programs[key] = _build_program(margin)
    return _programs[key]


def _softplus(x):
    return np.log1p(np.exp(-np.abs(x))) + np.maximum(x, 0)


def _make_in_maps(preds, labels, logits, targets, pos_weight, margin):
    m = float(margin)
    p = np.ascontiguousarray(np.asarray(preds, np.float32))
    l = np.ascontiguousarray(np.asarray(labels, np.float32))
    z = np.ascontiguousarray(np.asarray(logits, np.float32))
    tg = np.ascontiguousarray(np.asarray(targets, np.float32))

    g = _grid()
    lhsT = np.zeros((2, P), np.float64)
    lhsT[0, :] = 1.0
    lhsT[1, 0:G] = CMASK          # H / ZT coefficient columns
    lhsT[1, G:] = -CMASK          # F / Z coefficient columns

    t64 = np.zeros((G, G), np.float32)
    for k in range(1, G - 1):
        t64[k - 1, k] = 1.0
        t64[k, k] = -2.0
        t64[k + 1, k] = 1.0
    tmat = np.zeros((P, P), np.float32)
    tmat[0:NG * G, 0:NG * G] = np.tile(t64, (NG, NG))

    # both Z and ZT use H coeffs -> value z - g, knots g.
    # phi(z) = log(1+e^-z): the reference's log(e^-mv + e^-z-mv) + mv
    phi = np.zeros((P, 2), np.float32)
    phi[:, 0] = np.concatenate(
        [np.tile(np.log1p(np.exp(-g)), NG), np.zeros(P - NG * G)])
    phi[:, 1] = phi[:, 0]

    g2 = np.concatenate([np.tile(g, NG), np.zeros(P - NG * G)])
    biases = np.stack([-CMASK - g2, m - g2, -g2,
                       CMASK + g2, g2 - m, g2], axis=1).astype(np.float32)
    aux = np.concatenate([np.zeros((P, BCE_F), np.float32),
                          np.zeros((P, BCE_F), np.float32),
                          phi, biases], axis=1)

    ndt = ml_dtypes.bfloat16
    pb, lb = p.astype(ndt), l.astype(ndt)
    zb, tb = z.astype(ndt), tg.astype(ndt)
    in_maps = []
    for c in range(NCORES):
        sl = slice(SH * c, SH * (c + 1))
        rhs = np.zeros((2, P + HPAD + 2 * SH), ndt)
        rhs[:, 0:P] = lhsT.astype(ndt)
        o = P
        rhs[0, o:o + B] = pb          # H: p + C*l (+ bias_h); pads l=0
        rhs[1, o:o + B] = lb
        o = P + HPAD
        rhs[0, o:o + SH] = pb[sl]     # F: p - C*l  (+ bias_f)
        rhs[1, o:o + SH] = lb[sl]
        o = P + HPAD + SH
        rhs[0, o:] = zb[sl]           # ZT: z + C*t, consumed twice
        rhs[1, o:] = tb[sl]
        auxc = aux.copy()
        auxc[:, 0:BCE_F] = z[sl].reshape(P, BCE_F)
        auxc[:, BCE_F:2 * BCE_F] = tg[sl].reshape(P, BCE_F)
        in_maps.append({"rhs": rhs, "aux": auxc, "tmat": tmat})
    return in_maps


def _combine(outs: np.ndarray, margin: float, pos_weight) -> np.ndarray:
    # outs: [NCORES, 1, 5] partials
    # [dot(F_c, h*J), 2*h*n1, 2*h*SP1_c, 2*h*SPt_c, sum z(1-t)_c]
    m = float(margin)
    pw = float(np.asarray(pos_weight, np.float32).reshape(-1)[0])
    S = float(outs[:, 0, 0].sum()) / HSTEP
    n1 = float(outs[:, 0, 1].mean()) / (2.0 * HSTEP)
    n0 = B - n1
    sp_t0 = float(outs[:, 0, 2].sum()) / (NG * HSTEP)
    spt = float(outs[:, 0, 3].sum()) / (NG * HSTEP)
    # fold multiplicity: F/ZT pack 2 groups, n1 comes from the H column
    # folded NG ways -- rescale n1 by 2/NG
    n1 = n1 * 2.0 / NG
    n0 = B - n1
    zlin = float(outs[:, 0, 4].sum())
    rm = max(m, 0.0)
    margin_loss = ((n0 * n0 + n1 * n1) * rm + 2.0 * S) / (2.0 * B) - rm / 2.0
    bce_loss = (zlin + sp_t0 + pw * spt) / B
    return np.array([margin_loss, bce_loss], dtype=np.float32)


def _run(inputs: dict, trace: bool = False, **spmd_kwargs):
    m = float(np.asarray(inputs["margin"]))
    nc = _get_program(m)
    in_maps = _make_in_maps(inputs["preds"], inputs["labels"],
                            inputs["logits"], inputs["targets"],
                            inputs["pos_weight"], m)
    res = run_bass_kernel_spmd(nc, in_maps, core_ids=list(range(NCORES)),
                               trace=trace, **spmd_kwargs)
    outs = np.stack([np.asarray(r["out"], np.float32) for r in res.results])
    return _combine(outs, m, inputs["pos_weight"]), res


def kernel(preds, labels, logits, targets, pos_weight, margin):
    out, _ = _run(dict(preds=preds, labels=labels, logits=logits,
                       targets=targets, pos_weight=pos_weight,
                       margin=margin))
    return out

